# revision 30
# baseline (speedup 1.0000x reference)
"""Causal self-attention (RoPE) on 8 trn2 NeuronCores.

Sharding: tensor-parallel over heads; each core owns 2 of 16 heads.
Host sums the 8 partial projection outputs (the all-reduce) + bproj.

v4 design (vs v2, ~216us -> ~202us):
 - j-loop grouped by 2: scores for (j, j+1) into a 4-bank psum tile,
   one batched exp for both off-diagonal j (fewer ACT instructions),
   then the previous group's AV matmuls + 2 fill units keep the PE
   queue dense and mode-grouped (row-tiled scores vs full-array AV).
 - AV stationary is [keys, 65] (64 v cols + ones col) - denominator
   rides along as psum row 64; LDWEIGHTS drops from 128 to 65 cols.
 - Denominator drain is all fp16 (v2 used fp32 -> LOW_HIGH double-pass
   micro-matmuls): den rows, PE column transpose, fast column-shape
   reciprocal, PE broadcast matmuls.  drainq entries carry a fill-slot
   delay so drain matmuls never sit at the PE queue head waiting for
   evacuations that queue behind exp on ACT.
 - v wave split into two fill units (qkv matmuls + vn copy, then the
   PE transposes one slot later) so transposes never head-block.
 - Projection/v/den evacuations split across DVE and ACT; yT normalize
   multiply on GpSimd.  48 warmup matmuls cover the input-DMA window.
 (Tried and reverted, see memory: DMA-broadcast denominator paths are
 correct but stall engine FIFOs; SBUF-staged RoPE is illegal - base
 partitions must match unless one input is PSUM.)
"""

import ml_dtypes
import numpy as np

import concourse.bacc as bacc
import concourse.bass as bass
import concourse.mybir as mybir
import concourse.tile as tile
from concourse.bass_utils import run_bass_kernel_spmd

F32 = mybir.dt.float32
FP16 = mybir.dt.float16
BF16 = mybir.dt.bfloat16

B, T, C = 2, 2048, 1024
H, D = 16, 64
NCORES = 8
HL = 2                   # heads per core
R = B * T                # 4096 token rows
PB = 128
TBB = T // PB            # 16 row blocks per batch
QT = 512                 # attention query tile
NQT = T // QT            # 4 per batch
KC = C // PB             # 8 contraction chunks
NW = R // QT             # 8 qkv waves (one 512-token chunk each)
ROPE_BASE = 10000.0

MM_DT = BF16


def _build_nc(with_bias=False):
    nc = bacc.Bacc(trn_type="TRN2")

    xT = nc.dram_tensor("xT", [C, R], MM_DT, kind="ExternalInput")
    wq = nc.dram_tensor("wq", [C, 3 * HL * D], MM_DT, kind="ExternalInput")
    wp = nc.dram_tensor("wp", [HL * D, C], MM_DT, kind="ExternalInput")
    tbl = nc.dram_tensor("tbl", [PB, 2, T], MM_DT, kind="ExternalInput")
    idn = nc.dram_tensor("idn", [PB, PB], MM_DT, kind="ExternalInput")
    bm2 = nc.dram_tensor("bm2", [PB, 2 * PB], MM_DT, kind="ExternalInput")
    out = nc.dram_tensor("out", [R, C], MM_DT, kind="ExternalOutput")
    if with_bias:
        btbl = nc.dram_tensor("btbl", [PB, 2, T], F32, kind="ExternalInput")
        bv = nc.dram_tensor("bv", [PB, 1], F32, kind="ExternalInput")
    else:
        btbl = bv = None

    with tile.TileContext(nc) as tc:
        _body(nc, tc, xT, wq, wp, tbl, idn, bm2, out, btbl, bv)
    nc.finalize()
    return nc


def _body(nc, tc, xT, wq, wp, tbl, idn, bm2, out, btbl, bv):
    import contextlib

    ctx = contextlib.ExitStack()
    with ctx:
        singles = ctx.enter_context(tc.tile_pool(name="singles", bufs=1))

        # ---- resident constants -------------------------------------------
        KCB = [(0, 2), (2, 5), (5, 8)]
        wq_r = wq.rearrange("(kc p) n -> p kc n", p=PB)
        wq_p = []
        for lo, hi in KCB:
            t_ = singles.tile([PB, hi - lo, 3 * PB], MM_DT, name=f"wq{lo}")
            nc.scalar.dma_start(out=t_, in_=wq_r[:, lo:hi, :])
            wq_p.append(t_)

        def wq_at(kc, ncols):
            i = 0 if kc < 2 else (1 if kc < 5 else 2)
            return wq_p[i][:, kc - KCB[i][0], ncols]

        tbl_t = singles.tile([PB, 2, T], MM_DT)
        nc.gpsimd.dma_start(out=tbl_t, in_=tbl[:, :, :])
        idn_t = singles.tile([PB, PB], MM_DT)
        nc.gpsimd.dma_start(out=idn_t, in_=idn[:, :])
        bm_t = singles.tile([PB, 2, PB], MM_DT)
        wp_t = singles.tile([PB, C], MM_DT)
        if btbl is not None:
            btbl_t = singles.tile([PB, 2, T], F32)
            nc.gpsimd.dma_start(out=btbl_t, in_=btbl[:, :, :])
            bv_t = singles.tile([PB, 1], F32)
            nc.gpsimd.dma_start(out=bv_t, in_=bv[:, :])

        # ---- resident activations -----------------------------------------
        ones_h = singles.tile([PB, 1], FP16)
        nc.vector.memset(ones_h, 1.0)
        idn_h = singles.tile([PB, PB], FP16)
        nc.vector.tensor_copy(idn_h, idn_t)

        qkT_b = [
            singles.tile([PB, TBB, 2, PB], MM_DT, name=f"qkT{b}") for b in range(B)
        ]
        va_b = [
            singles.tile([PB, HL, TBB, PB], MM_DT, name=f"va{b}")
            for b in range(B)
        ]
        yT = singles.tile([PB, R], MM_DT)

        for b in range(B):
            # only column 64 (the denominator ones-column) needs init
            nc.gpsimd.memset(va_b[b][:, :, :, 64:65], 1.0)

        with (
            tc.tile_pool(name="xt", bufs=4) as xt_pool,
            tc.tile_pool(name="qn", bufs=2) as qn_pool,
            tc.tile_pool(name="dsb", bufs=2) as dsb_pool,
            tc.tile_pool(name="pt", bufs=3) as pt_pool,
            tc.tile_pool(name="ost", bufs=6) as ost_pool,
            tc.tile_pool(name="psq", bufs=2, space="PSUM") as psq_pool,
            tc.tile_pool(name="pss", bufs=1, space="PSUM") as pss_pool,
            tc.tile_pool(name="pso", bufs=1, space="PSUM") as pso_pool,
        ):
            xtt = {}

            xT_r = xT.rearrange("(kc p) t -> p kc t", p=PB)

            def prefetch(w, split=False):
                if w >= NW or w in xtt:
                    return
                cs = slice(w * QT, (w + 1) * QT)
                if split:
                    ps = []
                    for gi, (lo, hi) in enumerate(KCB):
                        t_ = xt_pool.tile([PB, hi - lo, QT], MM_DT,
                                          tag=f"xts{gi}", name=f"xt{w}_{gi}")
                        nc.sync.dma_start(out=t_, in_=xT_r[:, lo:hi, cs])
                        ps.append(t_)
                    xtt[w] = ps
                else:
                    t_ = xt_pool.tile([PB, KC, QT], MM_DT, tag="xt",
                                      name=f"xt{w}")
                    nc.sync.dma_start(out=t_, in_=xT_r[:, :, cs])
                    xtt[w] = t_

            def xt_at(w, kc):
                v = xtt[w]
                if isinstance(v, list):
                    i = 0 if kc < 2 else (1 if kc < 5 else 2)
                    return v[i][:, kc - KCB[i][0], :]
                return v[:, kc, :]

            # ---------------- qkv^T wave (one 512-token chunk) -------------
            vn_tiles = {}

            def wave_vtr(w):
                b, tc4 = divmod(w, NQT)
                tb0 = tc4 * 4
                vn = vn_tiles.pop(w)
                vtr = psq_pool.tile([PB, 4, PB], MM_DT, tag="psq", name="vtr")
                for i in range(4):
                    nc.tensor.transpose(
                        vtr[:, i, :], vn[:, i * PB : (i + 1) * PB], idn_t
                    )
                nc.vector.tensor_copy(
                    va_b[b][:, :, tb0 : tb0 + 4, 0:64],
                    vtr.rearrange("p i (h d) -> p h i d", h=2),
                )

            def wave_nblk(w, nblk):
                if nblk == 0:
                    prefetch(w + 3)
                b, tc4 = divmod(w, NQT)
                tb0 = tc4 * 4
                cols = slice(tc4 * QT, (tc4 + 1) * QT)   # within-batch t
                psq = psq_pool.tile([PB, QT], F32, tag="psq", name="psq")
                ncols = slice(nblk * PB, (nblk + 1) * PB)
                for kc in range(KC):
                    nc.tensor.matmul(
                        psq,
                        lhsT=wq_at(kc, ncols),
                        rhs=xt_at(w, kc),
                        start=(kc == 0),
                        stop=(kc == KC - 1),
                    )
                if nblk < 2:
                    # RoPE: partition-shifted reads are only legal with a
                    # PSUM input, so rt/qc read psq directly.
                    rt = qn_pool.tile([PB, QT], MM_DT, tag="rt", name="rt")
                    for qd in range(4):
                        ob, ib = qd * 32, (qd ^ 1) * 32
                        nc.vector.tensor_tensor(
                            out=rt[ob : ob + 32, :],
                            in0=psq[ib : ib + 32, :],
                            in1=tbl_t[ob : ob + 32, 1, cols],
                            op=mybir.AluOpType.mult,
                        )
                    qc = qn_pool.tile([PB, QT], MM_DT, tag="qc", name="qc")
                    nc.vector.tensor_tensor(
                        out=qc, in0=psq, in1=tbl_t[:, 0, cols],
                        op=mybir.AluOpType.mult,
                    )
                    dst = qkT_b[b][:, tb0 : tb0 + 4, nblk, :]
                    if btbl is None:
                        nc.gpsimd.tensor_tensor(
                            out=dst,
                            in0=qc.rearrange("p (a b) -> p a b", a=4),
                            in1=rt.rearrange("p (a b) -> p a b", a=4),
                            op=mybir.AluOpType.add,
                        )
                    else:
                        qr = qn_pool.tile([PB, QT], F32, tag="qr", name="qr")
                        nc.vector.tensor_tensor(
                            out=qr, in0=qc, in1=rt, op=mybir.AluOpType.add,
                        )
                        nc.vector.tensor_tensor(
                            out=dst,
                            in0=qr.rearrange("p (a b) -> p a b", a=4),
                            in1=btbl_t[:, nblk, cols].rearrange(
                                "p (a b) -> p a b", a=4
                            ),
                            op=mybir.AluOpType.add,
                        )
                else:
                    # v wave is split into two fill units so the PE-side
                    # transposes never wait at the queue head for the vn
                    # evacuation (unit 3 runs them a fill-slot later).
                    vn = qn_pool.tile([PB, QT], MM_DT, tag="vn", name="vn")
                    if btbl is None:
                        nc.vector.tensor_copy(vn, psq)
                    else:
                        nc.scalar.add(vn, psq, bv_t[:, 0:1])
                    vn_tiles[w] = vn

            # ---------------- filler machinery ------------------------------
            waveq = []
            drainq = []
            pending = []

            def emit_one_proj():
                if not pending:
                    return False
                qb = pending.pop(0)
                ot = ost_pool.tile([PB, C], MM_DT, tag="ot", name="ot")
                for nch in range(2):
                    pp = psq_pool.tile([PB, QT], F32, tag="psq", name="pp")
                    nc.tensor.matmul(
                        pp,
                        lhsT=yT[:, qb * PB : (qb + 1) * PB],
                        rhs=wp_t[:, nch * QT : (nch + 1) * QT],
                        start=True,
                        stop=True,
                    )
                    if nch == 0:
                        nc.vector.tensor_copy(ot[:, 0:QT], pp)
                    else:
                        nc.scalar.copy(ot[:, QT:C], pp)
                nc.sync.dma_start(out=out[qb * PB : (qb + 1) * PB, :], in_=ot)
                return True

            def emit_fill():
                # drainq entries are [delay, closure]: delay counts fill
                # slots before the closure may run (lets the den DMA chain
                # land before the norm multiply hits the DVE queue head).
                if drainq and drainq[0][0] <= 0:
                    drainq.pop(0)[1]()
                    return
                if drainq:
                    drainq[0][0] -= 1
                if waveq:
                    waveq.pop(0)[1]()
                else:
                    emit_one_proj()

            def queue_wave(w):
                for nblk in range(3):
                    waveq.append((w, lambda w=w, n=nblk: wave_nblk(w, n)))
                waveq.append((w, lambda w=w: wave_vtr(w)))

            def flush_waves(k):
                while waveq and waveq[0][0] <= k:
                    waveq.pop(0)[1]()

            # ---------------- attention for one query tile -----------------
            def attn_qt(b, qt):
                po = pso_pool.tile([PB, HL, QT], F32, tag="po", name="po")
                jmax = qt * 4 + 4

                def s_off(j):
                    return max(j - qt * 4, 0) * PB

                def make_av(ja, jb, pt):
                    def av():
                        for par, j in ((0, ja), (1, jb)):
                            off = s_off(j)
                            for h in range(HL):
                                nc.tensor.matmul(
                                    po[0:65, h, off:QT],
                                    lhsT=va_b[b][:, h, j, 0:65],
                                    rhs=pt[:, h, par, off:QT],
                                    start=(j == 0),
                                    stop=(j == jmax - 1),
                                )
                    return av

                prev_av = None
                for g0 in range(0, jmax, 2):
                    ja, jb = g0, g0 + 1
                    ps = pss_pool.tile([PB, HL, 2, QT], F32, tag="pss",
                                       name="ps")
                    pt = pt_pool.tile([PB, HL, 2, QT], MM_DT, tag="pt",
                                      name="pt")
                    for par, j in ((0, ja), (1, jb)):
                        off = s_off(j)
                        for h in range(HL):
                            nc.tensor.matmul(
                                ps[:, h, par, off:QT],
                                lhsT=qkT_b[b][h * 64 : h * 64 + 64, j, 1, :],
                                rhs=qkT_b[b][
                                    h * 64 : h * 64 + 64,
                                    qt * 4 + off // PB : qt * 4 + 4, 0, :,
                                ],
                                start=True,
                                stop=True,
                            )
                    offa, offb = s_off(ja), s_off(jb)
                    if offa == 0 and offb == 0:
                        nc.scalar.activation(
                            out=pt, in_=ps,
                            func=mybir.ActivationFunctionType.Exp, scale=0.125,
                        )
                    else:
                        for par, j in ((0, ja), (1, jb)):
                            off = s_off(j)
                            nc.scalar.activation(
                                out=pt[:, :, par, off:QT],
                                in_=ps[:, :, par, off:QT],
                                func=mybir.ActivationFunctionType.Exp,
                                scale=0.125,
                            )
                    for par, j in ((0, ja), (1, jb)):
                        off = s_off(j)
                        if j - qt * 4 >= 0:
                            nc.gpsimd.tensor_tensor(
                                out=pt[:, :, par, off : off + PB],
                                in0=pt[:, :, par, off : off + PB],
                                in1=bm_t, op=mybir.AluOpType.mult,
                            )
                    if prev_av is not None:
                        prev_av()
                    emit_fill()
                    emit_fill()
                    prev_av = make_av(ja, jb, pt)
                prev_av()

                # ---- release po fast: den row + raw y^T copies -------------
                # (fp16 den -> single-pass PE transpose + broadcast matmuls;
                # a DMA-based broadcast stalls the engine FIFOs, measured.)
                cols = slice(b * T + qt * QT, b * T + (qt + 1) * QT)
                den = dsb_pool.tile([65, HL, QT], FP16, tag="den", name="den")
                with nc.allow_low_precision(reason="fp16 den rows"):
                    nc.scalar.copy(den[64:65, 0, :], po[64:65, 0, :])
                    nc.vector.tensor_copy(den[64:65, 1, :], po[64:65, 1, :])
                yraw = dsb_pool.tile([PB, QT], MM_DT, tag="yraw", name="yraw")
                nc.vector.tensor_copy(yraw[0:64, :], po[0:64, 0, :])
                nc.scalar.copy(yraw[64:128, :], po[0:64, 1, :])

                def drain1(den=den, b=b, qt=qt, yraw=yraw):
                    # den rows -> columns (PE), fast column-shape reciprocal,
                    # replicate reciprocal columns for the broadcast matmul.
                    dtr = psq_pool.tile([PB, HL, 4], F32, tag="psq",
                                        name="dtr")
                    for h in range(HL):
                        for i in range(4):
                            nc.tensor.matmul(
                                dtr[:, h, i : i + 1],
                                lhsT=den[64:65, h, i * PB : (i + 1) * PB],
                                rhs=ones_h[64:65, 0:1],
                                start=True,
                                stop=True,
                                tile_position=(64, 0),
                            )
                    rec = dsb_pool.tile([PB, HL, 4], FP16, tag="rec",
                                        name="rec")
                    with nc.allow_low_precision(reason="fp16 recip of den"):
                        nc.vector.reciprocal(rec, dtr)
                    rec2 = dsb_pool.tile([PB, HL, 4, 64], FP16, tag="rec2",
                                         name="rec2")
                    nc.vector.tensor_copy(
                        rec2,
                        bass.AP(tensor=rec.tensor, offset=rec.offset,
                                ap=[list(rec.ap[0]), [4, 2], [1, 4], [0, 64]]),
                    )
                    for i in range(4):
                        drainq.append([1, lambda i=i: drain2(rec2, i)])

                def drain2(rec2, i, b=b, qt=qt, yraw=yraw):
                    # broadcast per-head reciprocal and normalize one
                    # 128-query block of y^T, then queue its projection.
                    qb = (b * T + qt * QT) // PB + i
                    rn = psq_pool.tile([PB, PB], F32, tag="psq", name="rn")
                    for h in range(HL):
                        nc.tensor.matmul(
                            rn[h * 64 : h * 64 + 64, :],
                            lhsT=rec2[:, h, i, :],
                            rhs=idn_h,
                            start=True,
                            stop=True,
                        )
                    rn_sb = dsb_pool.tile([PB, PB], FP16, tag=f"rn{i % 2}",
                                          name="rn_sb")
                    nc.vector.tensor_copy(rn_sb, rn)
                    nc.gpsimd.tensor_tensor(
                        out=yT[:, qb * PB : (qb + 1) * PB],
                        in0=yraw[:, i * PB : (i + 1) * PB], in1=rn_sb,
                        op=mybir.AluOpType.mult,
                    )
                    pending.append(qb)

                drainq.append([3, drain1])

            # ---------------- schedule -------------------------------------
            prefetch(0, split=True)
            prefetch(1, split=True)
            prefetch(2)
            # PE warmup: dummy matmuls during the DMA-init window keep the
            # HAM clock-gate at full rate for the first real wave.
            wz = qn_pool.tile([PB, PB], MM_DT, tag="rt", name="warm")
            nc.vector.memset(wz, 0.0)
            pw = psq_pool.tile([PB, QT], F32, tag="psq", name="pw")
            for i in range(88):
                nc.tensor.matmul(pw[:, 0:PB], lhsT=wz, rhs=wz,
                                 start=True, stop=True)
            nc.gpsimd.dma_start(out=bm_t,
                                in_=bm2.rearrange("p (g x) -> p g x", g=2))
            nc.gpsimd.dma_start(out=wp_t, in_=wp[:, :])
            for w in range(NW):
                queue_wave(w)
            for b in range(B):
                for qt in range(NQT):
                    flush_waves(b * NQT + qt)
                    attn_qt(b, qt)
            while drainq or waveq or pending:
                if waveq:
                    waveq.pop(0)[1]()
                if drainq:
                    drainq.pop(0)[1]()
                emit_one_proj()


_NC_CACHE = {}
LAST_RESULTS = None


def _tables():
    inv = 1.0 / (ROPE_BASE ** (np.arange(0, D, 2, dtype=np.float32) / D))
    t = np.arange(T, dtype=np.float32)
    fr = np.einsum("i,j->ij", t, inv)             # [T, 32]
    emb = np.concatenate([fr, fr], axis=1)        # [T, 64]
    cos = np.cos(emb).T.astype(np.float32)        # [64, T]
    sin = np.sin(emb).T.astype(np.float32)
    sgn = np.concatenate([-sin[0:32], sin[32:64]], axis=0)   # [64, T]
    cos2 = np.concatenate([cos, cos], axis=0)     # [128, T] two heads
    sgn2 = np.concatenate([sgn, sgn], axis=0)
    tbl = np.stack([cos2, sgn2], axis=1)          # [128, 2, T]
    return np.ascontiguousarray(tbl).astype(ml_dtypes.bfloat16)


def kernel(x, Wqkv, bqkv, Wproj, bproj):
    global LAST_RESULTS
    x = np.asarray(x, dtype=np.float32)
    Wqkv = np.asarray(Wqkv, dtype=np.float32)
    bqkv = np.asarray(bqkv, dtype=np.float32)
    Wproj = np.asarray(Wproj, dtype=np.float32)
    bproj = np.asarray(bproj, dtype=np.float32)
    with_bias = bool(np.any(bqkv))

    xT = np.ascontiguousarray(x.reshape(R, C).T).astype(ml_dtypes.bfloat16)
    tbl = _tables()
    idn = np.eye(PB, dtype=np.float32).astype(ml_dtypes.bfloat16)
    bm0 = (np.tril(np.ones((PB, PB), dtype=np.float32))).T  # [k,u]: u>=k
    bm2 = np.ascontiguousarray(np.concatenate([bm0, bm0], axis=1)).astype(
        ml_dtypes.bfloat16
    )

    in_maps = []
    for r in range(NCORES):
        hsel = [2 * r, 2 * r + 1]
        wcols = []
        for part in range(3):  # q, k, v column groups
            for h in hsel:
                wcols.append(Wqkv[:, part * C + h * D : part * C + (h + 1) * D])
        wq_l = np.ascontiguousarray(np.concatenate(wcols, axis=1)).astype(
            ml_dtypes.bfloat16
        )
        wp_l = np.ascontiguousarray(Wproj[r * PB : (r + 1) * PB, :]).astype(
            ml_dtypes.bfloat16
        )
        m = {"xT": xT, "wq": wq_l, "wp": wp_l, "tbl": tbl, "idn": idn,
             "bm2": bm2}
        if with_bias:
            bq_cols = []
            for part in range(3):
                for h in hsel:
                    bq_cols.append(
                        bqkv[part * C + h * D : part * C + (h + 1) * D]
                    )
            bq_l = np.stack([np.concatenate(bq_cols[0:2]),
                             np.concatenate(bq_cols[2:4])])   # [2, 128] q,k
            bvv = np.concatenate(bq_cols[4:6]).reshape(PB, 1)  # [128,1] v

            def rot(vec):
                o = np.empty_like(vec)
                for base in (0, 64):
                    o[base:base + 32] = -vec[base + 32:base + 64]
                    o[base + 32:base + 64] = vec[base:base + 32]
                return o

            inv = 1.0 / (ROPE_BASE ** (np.arange(0, D, 2) / D))
            tt = np.arange(T, dtype=np.float32)
            fr = np.einsum("i,j->ij", tt, inv)
            emb = np.concatenate([fr, fr], axis=1)
            cosf = np.concatenate([np.cos(emb).T] * 2, axis=0).astype(np.float32)
            sinf = np.concatenate([np.sin(emb).T] * 2, axis=0).astype(np.float32)
            bt = np.empty((PB, 2, T), np.float32)
            for g in range(2):
                bcol = bq_l[g][:, None].astype(np.float32)
                bt[:, g, :] = bcol * cosf + rot(bcol[:, 0])[:, None] * sinf
            m["btbl"] = bt
            m["bv"] = np.ascontiguousarray(bvv.astype(np.float32))
        in_maps.append(m)

    key = with_bias
    if key not in _NC_CACHE:
        _NC_CACHE[key] = _build_nc(with_bias)
    res = run_bass_kernel_spmd(_NC_CACHE[key], in_maps,
                               core_ids=list(range(NCORES)))
    LAST_RESULTS = res
    acc = np.zeros((R, C), dtype=np.float32)
    for r in range(NCORES):
        acc += res.results[r]["out"].astype(np.float32)
    acc += bproj[None, :]
    return acc.reshape(B, T, C)


# revision 31
# speedup vs baseline: 1.0162x; 1.0162x over previous
"""Causal self-attention (RoPE) on 8 trn2 NeuronCores.

Sharding: tensor-parallel over heads; each core owns 2 of 16 heads.
Host sums the 8 partial projection outputs (the all-reduce) + bproj.

v4 design (vs v2, ~216us -> ~202us):
 - j-loop grouped by 2: scores for (j, j+1) into a 4-bank psum tile,
   one batched exp for both off-diagonal j (fewer ACT instructions),
   then the previous group's AV matmuls + 2 fill units keep the PE
   queue dense and mode-grouped (row-tiled scores vs full-array AV).
 - AV stationary is [keys, 65] (64 v cols + ones col) - denominator
   rides along as psum row 64; LDWEIGHTS drops from 128 to 65 cols.
 - Denominator drain is all fp16 (v2 used fp32 -> LOW_HIGH double-pass
   micro-matmuls): den rows, PE column transpose, fast column-shape
   reciprocal, PE broadcast matmuls.  drainq entries carry a fill-slot
   delay so drain matmuls never sit at the PE queue head waiting for
   evacuations that queue behind exp on ACT.
 - v wave split into two fill units (qkv matmuls + vn copy, then the
   PE transposes one slot later) so transposes never head-block.
 - Projection/v/den evacuations split across DVE and ACT; yT normalize
   multiply on GpSimd.  48 warmup matmuls cover the input-DMA window.
 (Tried and reverted, see memory: DMA-broadcast denominator paths are
 correct but stall engine FIFOs; SBUF-staged RoPE is illegal - base
 partitions must match unless one input is PSUM.)
"""

import ml_dtypes
import numpy as np

import concourse.bacc as bacc
import concourse.bass as bass
import concourse.mybir as mybir
import concourse.tile as tile
from concourse.bass_utils import run_bass_kernel_spmd

F32 = mybir.dt.float32
FP16 = mybir.dt.float16
BF16 = mybir.dt.bfloat16

B, T, C = 2, 2048, 1024
H, D = 16, 64
NCORES = 8
HL = 2                   # heads per core
R = B * T                # 4096 token rows
PB = 128
TBB = T // PB            # 16 row blocks per batch
QT = 512                 # attention query tile
NQT = T // QT            # 4 per batch
KC = C // PB             # 8 contraction chunks
NW = R // QT             # 8 qkv waves (one 512-token chunk each)
ROPE_BASE = 10000.0

MM_DT = BF16


def _build_nc(with_bias=False):
    nc = bacc.Bacc(trn_type="TRN2")

    xT = nc.dram_tensor("xT", [C, R], MM_DT, kind="ExternalInput")
    wq = nc.dram_tensor("wq", [C, 3 * HL * D], MM_DT, kind="ExternalInput")
    wp = nc.dram_tensor("wp", [HL * D, C], MM_DT, kind="ExternalInput")
    tbl = nc.dram_tensor("tbl", [PB, 2, T], MM_DT, kind="ExternalInput")
    idn = nc.dram_tensor("idn", [PB, PB], MM_DT, kind="ExternalInput")
    bm2 = nc.dram_tensor("bm2", [PB, 2 * PB], MM_DT, kind="ExternalInput")
    out = nc.dram_tensor("out", [R, C], MM_DT, kind="ExternalOutput")
    if with_bias:
        btbl = nc.dram_tensor("btbl", [PB, 2, T], F32, kind="ExternalInput")
        bv = nc.dram_tensor("bv", [PB, 1], F32, kind="ExternalInput")
    else:
        btbl = bv = None

    with tile.TileContext(nc) as tc:
        _body(nc, tc, xT, wq, wp, tbl, idn, bm2, out, btbl, bv)
    nc.finalize()
    return nc


def _body(nc, tc, xT, wq, wp, tbl, idn, bm2, out, btbl, bv):
    import contextlib

    ctx = contextlib.ExitStack()
    with ctx:
        singles = ctx.enter_context(tc.tile_pool(name="singles", bufs=1))

        # ---- resident constants -------------------------------------------
        KCB = [(0, 2), (2, 5), (5, 8)]
        wq_r = wq.rearrange("(kc p) n -> p kc n", p=PB)
        wq_p = []
        for lo, hi in KCB:
            t_ = singles.tile([PB, hi - lo, 3 * PB], MM_DT, name=f"wq{lo}")
            nc.scalar.dma_start(out=t_, in_=wq_r[:, lo:hi, :])
            wq_p.append(t_)

        def wq_at(kc, ncols):
            i = 0 if kc < 2 else (1 if kc < 5 else 2)
            return wq_p[i][:, kc - KCB[i][0], ncols]

        tbl_t = singles.tile([PB, 2, T], MM_DT)
        nc.gpsimd.dma_start(out=tbl_t, in_=tbl[:, :, :])
        idn_t = singles.tile([PB, PB], MM_DT)
        nc.gpsimd.dma_start(out=idn_t, in_=idn[:, :])
        bm_t = singles.tile([PB, 2, PB], MM_DT)
        wp_t = singles.tile([PB, C], MM_DT)
        if btbl is not None:
            btbl_t = singles.tile([PB, 2, T], F32)
            nc.gpsimd.dma_start(out=btbl_t, in_=btbl[:, :, :])
            bv_t = singles.tile([PB, 1], F32)
            nc.gpsimd.dma_start(out=bv_t, in_=bv[:, :])

        # ---- resident activations -----------------------------------------
        ones_h = singles.tile([PB, 1], FP16)
        nc.vector.memset(ones_h, 1.0)
        idn_h = singles.tile([PB, PB], FP16)
        nc.vector.tensor_copy(idn_h, idn_t)

        qkT_b = [
            singles.tile([PB, TBB, 2, PB], MM_DT, name=f"qkT{b}") for b in range(B)
        ]
        va_b = [
            singles.tile([PB, HL, TBB, PB], MM_DT, name=f"va{b}")
            for b in range(B)
        ]
        yT = singles.tile([PB, R], MM_DT)

        for b in range(B):
            # only column 64 (the denominator ones-column) needs init
            nc.gpsimd.memset(va_b[b][:, :, :, 64:65], 1.0)

        with (
            tc.tile_pool(name="xt", bufs=4) as xt_pool,
            tc.tile_pool(name="qn", bufs=2) as qn_pool,
            tc.tile_pool(name="dsb", bufs=2) as dsb_pool,
            tc.tile_pool(name="pt", bufs=3) as pt_pool,
            tc.tile_pool(name="ost", bufs=6) as ost_pool,
            tc.tile_pool(name="psq", bufs=2, space="PSUM") as psq_pool,
            tc.tile_pool(name="pss", bufs=1, space="PSUM") as pss_pool,
            tc.tile_pool(name="pso", bufs=1, space="PSUM") as pso_pool,
        ):
            xtt = {}

            xT_r = xT.rearrange("(kc p) t -> p kc t", p=PB)

            def prefetch(w, split=False):
                if w >= NW or w in xtt:
                    return
                cs = slice(w * QT, (w + 1) * QT)
                if split:
                    ps = []
                    for gi, (lo, hi) in enumerate(KCB):
                        t_ = xt_pool.tile([PB, hi - lo, QT], MM_DT,
                                          tag=f"xts{gi}", name=f"xt{w}_{gi}")
                        nc.sync.dma_start(out=t_, in_=xT_r[:, lo:hi, cs])
                        ps.append(t_)
                    xtt[w] = ps
                else:
                    t_ = xt_pool.tile([PB, KC, QT], MM_DT, tag="xt",
                                      name=f"xt{w}")
                    nc.sync.dma_start(out=t_, in_=xT_r[:, :, cs])
                    xtt[w] = t_

            def xt_at(w, kc):
                v = xtt[w]
                if isinstance(v, list):
                    i = 0 if kc < 2 else (1 if kc < 5 else 2)
                    return v[i][:, kc - KCB[i][0], :]
                return v[:, kc, :]

            # ---------------- qkv^T wave (one 512-token chunk) -------------
            vn_tiles = {}

            def wave_vtr(w):
                b, tc4 = divmod(w, NQT)
                tb0 = tc4 * 4
                vn = vn_tiles.pop(w)
                vtr = psq_pool.tile([PB, 4, PB], MM_DT, tag="psq", name="vtr")
                for i in range(4):
                    nc.tensor.transpose(
                        vtr[:, i, :], vn[:, i * PB : (i + 1) * PB], idn_t
                    )
                nc.vector.tensor_copy(
                    va_b[b][:, :, tb0 : tb0 + 4, 0:64],
                    vtr.rearrange("p i (h d) -> p h i d", h=2),
                )

            def wave_nblk(w, nblk):
                if nblk == 0:
                    prefetch(w + 3)
                b, tc4 = divmod(w, NQT)
                tb0 = tc4 * 4
                cols = slice(tc4 * QT, (tc4 + 1) * QT)   # within-batch t
                psq = psq_pool.tile([PB, QT], F32, tag="psq", name="psq")
                ncols = slice(nblk * PB, (nblk + 1) * PB)
                for kc in range(KC):
                    nc.tensor.matmul(
                        psq,
                        lhsT=wq_at(kc, ncols),
                        rhs=xt_at(w, kc),
                        start=(kc == 0),
                        stop=(kc == KC - 1),
                    )
                if nblk < 2:
                    # RoPE: partition-shifted reads are only legal with a
                    # PSUM input, so rt/qc read psq directly.
                    rt = qn_pool.tile([PB, QT], MM_DT, tag="rt", name="rt")
                    for qd in range(4):
                        ob, ib = qd * 32, (qd ^ 1) * 32
                        nc.vector.tensor_tensor(
                            out=rt[ob : ob + 32, :],
                            in0=psq[ib : ib + 32, :],
                            in1=tbl_t[ob : ob + 32, 1, cols],
                            op=mybir.AluOpType.mult,
                        )
                    qc = qn_pool.tile([PB, QT], MM_DT, tag="qc", name="qc")
                    nc.vector.tensor_tensor(
                        out=qc, in0=psq, in1=tbl_t[:, 0, cols],
                        op=mybir.AluOpType.mult,
                    )
                    dst = qkT_b[b][:, tb0 : tb0 + 4, nblk, :]
                    if btbl is None:
                        nc.gpsimd.tensor_tensor(
                            out=dst,
                            in0=qc.rearrange("p (a b) -> p a b", a=4),
                            in1=rt.rearrange("p (a b) -> p a b", a=4),
                            op=mybir.AluOpType.add,
                        )
                    else:
                        qr = qn_pool.tile([PB, QT], F32, tag="qr", name="qr")
                        nc.vector.tensor_tensor(
                            out=qr, in0=qc, in1=rt, op=mybir.AluOpType.add,
                        )
                        nc.vector.tensor_tensor(
                            out=dst,
                            in0=qr.rearrange("p (a b) -> p a b", a=4),
                            in1=btbl_t[:, nblk, cols].rearrange(
                                "p (a b) -> p a b", a=4
                            ),
                            op=mybir.AluOpType.add,
                        )
                else:
                    # v wave is split into two fill units so the PE-side
                    # transposes never wait at the queue head for the vn
                    # evacuation (unit 3 runs them a fill-slot later).
                    vn = qn_pool.tile([PB, QT], MM_DT, tag="vn", name="vn")
                    if btbl is None:
                        nc.vector.tensor_copy(vn, psq)
                    else:
                        nc.scalar.add(vn, psq, bv_t[:, 0:1])
                    vn_tiles[w] = vn

            # ---------------- filler machinery ------------------------------
            waveq = []
            drainq = []
            pending = []

            def emit_one_proj():
                if not pending:
                    return False
                qb = pending.pop(0)
                ot = ost_pool.tile([PB, C], MM_DT, tag="ot", name="ot")
                for nch in range(2):
                    pp = psq_pool.tile([PB, QT], F32, tag="psq", name="pp")
                    nc.tensor.matmul(
                        pp,
                        lhsT=yT[:, qb * PB : (qb + 1) * PB],
                        rhs=wp_t[:, nch * QT : (nch + 1) * QT],
                        start=True,
                        stop=True,
                    )
                    if nch == 0:
                        nc.vector.tensor_copy(ot[:, 0:QT], pp)
                    else:
                        nc.scalar.copy(ot[:, QT:C], pp)
                nc.sync.dma_start(out=out[qb * PB : (qb + 1) * PB, :], in_=ot)
                return True

            def emit_fill():
                # drainq entries are [delay, closure]: delay counts fill
                # slots before the closure may run (lets the den DMA chain
                # land before the norm multiply hits the DVE queue head).
                if drainq and drainq[0][0] <= 0:
                    drainq.pop(0)[1]()
                    return
                if drainq:
                    drainq[0][0] -= 1
                if waveq:
                    waveq.pop(0)[1]()
                else:
                    emit_one_proj()

            def queue_wave(w):
                for nblk in range(3):
                    waveq.append((w, lambda w=w, n=nblk: wave_nblk(w, n)))
                waveq.append((w, lambda w=w: wave_vtr(w)))

            def flush_waves(k):
                while waveq and waveq[0][0] <= k:
                    waveq.pop(0)[1]()

            # ---------------- attention for one query tile -----------------
            def attn_qt(b, qt):
                po = pso_pool.tile([PB, HL, QT], F32, tag="po", name="po")
                jmax = qt * 4 + 4

                def s_off(j):
                    return max(j - qt * 4, 0) * PB

                def make_av(ja, jb, pt):
                    def av():
                        for par, j in ((0, ja), (1, jb)):
                            off = s_off(j)
                            for h in range(HL):
                                nc.tensor.matmul(
                                    po[0:65, h, off:QT],
                                    lhsT=va_b[b][:, h, j, 0:65],
                                    rhs=pt[:, h, par, off:QT],
                                    start=(j == 0),
                                    stop=(j == jmax - 1),
                                )
                    return av

                prev_av = None
                for g0 in range(0, jmax, 2):
                    ja, jb = g0, g0 + 1
                    ps = pss_pool.tile([PB, HL, 2, QT], F32, tag="pss",
                                       name="ps")
                    pt = pt_pool.tile([PB, HL, 2, QT], MM_DT, tag="pt",
                                      name="pt")
                    for par, j in ((0, ja), (1, jb)):
                        off = s_off(j)
                        for h in range(HL):
                            nc.tensor.matmul(
                                ps[:, h, par, off:QT],
                                lhsT=qkT_b[b][h * 64 : h * 64 + 64, j, 1, :],
                                rhs=qkT_b[b][
                                    h * 64 : h * 64 + 64,
                                    qt * 4 + off // PB : qt * 4 + 4, 0, :,
                                ],
                                start=True,
                                stop=True,
                            )
                    offa, offb = s_off(ja), s_off(jb)
                    if offa == 0 and offb == 0:
                        nc.scalar.activation(
                            out=pt, in_=ps,
                            func=mybir.ActivationFunctionType.Exp, scale=0.125,
                        )
                    else:
                        for par, j in ((0, ja), (1, jb)):
                            off = s_off(j)
                            nc.scalar.activation(
                                out=pt[:, :, par, off:QT],
                                in_=ps[:, :, par, off:QT],
                                func=mybir.ActivationFunctionType.Exp,
                                scale=0.125,
                            )
                    for par, j in ((0, ja), (1, jb)):
                        off = s_off(j)
                        if j - qt * 4 >= 0:
                            nc.gpsimd.tensor_tensor(
                                out=pt[:, :, par, off : off + PB],
                                in0=pt[:, :, par, off : off + PB],
                                in1=bm_t, op=mybir.AluOpType.mult,
                            )
                    if prev_av is not None:
                        prev_av()
                    emit_fill()
                    emit_fill()
                    prev_av = make_av(ja, jb, pt)
                prev_av()

                # ---- release po fast: den row + raw y^T copies -------------
                # (fp16 den -> single-pass PE transpose + broadcast matmuls;
                # a DMA-based broadcast stalls the engine FIFOs, measured.)
                cols = slice(b * T + qt * QT, b * T + (qt + 1) * QT)
                den = dsb_pool.tile([65, HL, QT], FP16, tag="den", name="den")
                with nc.allow_low_precision(reason="fp16 den rows"):
                    nc.scalar.copy(den[64:65, 0, :], po[64:65, 0, :])
                    nc.vector.tensor_copy(den[64:65, 1, :], po[64:65, 1, :])
                yraw = dsb_pool.tile([PB, QT], MM_DT, tag="yraw", name="yraw")
                nc.vector.tensor_copy(yraw[0:64, :], po[0:64, 0, :])
                nc.scalar.copy(yraw[64:128, :], po[0:64, 1, :])

                def drain1(den=den, b=b, qt=qt, yraw=yraw):
                    # den rows -> columns (PE), fast column-shape reciprocal,
                    # replicate reciprocal columns for the broadcast matmul.
                    dtr = psq_pool.tile([PB, HL, 4], F32, tag="psq",
                                        name="dtr")
                    for h in range(HL):
                        for i in range(4):
                            nc.tensor.matmul(
                                dtr[:, h, i : i + 1],
                                lhsT=den[64:65, h, i * PB : (i + 1) * PB],
                                rhs=ones_h[64:65, 0:1],
                                start=True,
                                stop=True,
                                tile_position=(64, 0),
                            )
                    rec = dsb_pool.tile([PB, HL, 4], FP16, tag="rec",
                                        name="rec")
                    with nc.allow_low_precision(reason="fp16 recip of den"):
                        nc.vector.reciprocal(rec, dtr)
                    rec2 = dsb_pool.tile([PB, HL, 4, 64], FP16, tag="rec2",
                                         name="rec2")
                    nc.vector.tensor_copy(
                        rec2,
                        bass.AP(tensor=rec.tensor, offset=rec.offset,
                                ap=[list(rec.ap[0]), [4, 2], [1, 4], [0, 64]]),
                    )
                    for i in range(4):
                        drainq.append([1, lambda i=i: drain2(rec2, i)])

                def drain2(rec2, i, b=b, qt=qt, yraw=yraw):
                    # broadcast per-head reciprocal and normalize one
                    # 128-query block of y^T, then queue its projection.
                    qb = (b * T + qt * QT) // PB + i
                    rn = psq_pool.tile([PB, PB], F32, tag="psq", name="rn")
                    for h in range(HL):
                        nc.tensor.matmul(
                            rn[h * 64 : h * 64 + 64, :],
                            lhsT=rec2[:, h, i, :],
                            rhs=idn_h,
                            start=True,
                            stop=True,
                        )
                    rn_sb = dsb_pool.tile([PB, PB], FP16, tag=f"rn{i % 2}",
                                          name="rn_sb")
                    nc.vector.tensor_copy(rn_sb, rn)
                    nc.gpsimd.tensor_tensor(
                        out=yT[:, qb * PB : (qb + 1) * PB],
                        in0=yraw[:, i * PB : (i + 1) * PB], in1=rn_sb,
                        op=mybir.AluOpType.mult,
                    )
                    pending.append(qb)

                drainq.append([3, drain1])

            # ---------------- schedule -------------------------------------
            prefetch(0, split=True)
            prefetch(1, split=True)
            prefetch(2)
            # PE warmup: dummy matmuls during the DMA-init window keep the
            # HAM clock-gate at full rate for the first real wave.
            wz = qn_pool.tile([PB, PB], MM_DT, tag="rt", name="warm")
            nc.vector.memset(wz, 0.0)
            pw = psq_pool.tile([PB, QT], F32, tag="psq", name="pw")
            for i in range(48):
                nc.tensor.matmul(pw[:, 0:PB], lhsT=wz, rhs=wz,
                                 start=True, stop=True)
            nc.gpsimd.dma_start(out=bm_t,
                                in_=bm2.rearrange("p (g x) -> p g x", g=2))
            nc.gpsimd.dma_start(out=wp_t, in_=wp[:, :])
            for w in range(NW):
                queue_wave(w)
            for b in range(B):
                for qt in range(NQT):
                    flush_waves(b * NQT + qt)
                    attn_qt(b, qt)
            while drainq or waveq or pending:
                if waveq:
                    waveq.pop(0)[1]()
                if drainq:
                    drainq.pop(0)[1]()
                emit_one_proj()


_NC_CACHE = {}
LAST_RESULTS = None


def _tables():
    inv = 1.0 / (ROPE_BASE ** (np.arange(0, D, 2, dtype=np.float32) / D))
    t = np.arange(T, dtype=np.float32)
    fr = np.einsum("i,j->ij", t, inv)             # [T, 32]
    emb = np.concatenate([fr, fr], axis=1)        # [T, 64]
    cos = np.cos(emb).T.astype(np.float32)        # [64, T]
    sin = np.sin(emb).T.astype(np.float32)
    sgn = np.concatenate([-sin[0:32], sin[32:64]], axis=0)   # [64, T]
    cos2 = np.concatenate([cos, cos], axis=0)     # [128, T] two heads
    sgn2 = np.concatenate([sgn, sgn], axis=0)
    tbl = np.stack([cos2, sgn2], axis=1)          # [128, 2, T]
    return np.ascontiguousarray(tbl).astype(ml_dtypes.bfloat16)


def kernel(x, Wqkv, bqkv, Wproj, bproj):
    global LAST_RESULTS
    x = np.asarray(x, dtype=np.float32)
    Wqkv = np.asarray(Wqkv, dtype=np.float32)
    bqkv = np.asarray(bqkv, dtype=np.float32)
    Wproj = np.asarray(Wproj, dtype=np.float32)
    bproj = np.asarray(bproj, dtype=np.float32)
    with_bias = bool(np.any(bqkv))

    xT = np.ascontiguousarray(x.reshape(R, C).T).astype(ml_dtypes.bfloat16)
    tbl = _tables()
    idn = np.eye(PB, dtype=np.float32).astype(ml_dtypes.bfloat16)
    bm0 = (np.tril(np.ones((PB, PB), dtype=np.float32))).T  # [k,u]: u>=k
    bm2 = np.ascontiguousarray(np.concatenate([bm0, bm0], axis=1)).astype(
        ml_dtypes.bfloat16
    )

    in_maps = []
    for r in range(NCORES):
        hsel = [2 * r, 2 * r + 1]
        wcols = []
        for part in range(3):  # q, k, v column groups
            for h in hsel:
                wcols.append(Wqkv[:, part * C + h * D : part * C + (h + 1) * D])
        wq_l = np.ascontiguousarray(np.concatenate(wcols, axis=1)).astype(
            ml_dtypes.bfloat16
        )
        wp_l = np.ascontiguousarray(Wproj[r * PB : (r + 1) * PB, :]).astype(
            ml_dtypes.bfloat16
        )
        m = {"xT": xT, "wq": wq_l, "wp": wp_l, "tbl": tbl, "idn": idn,
             "bm2": bm2}
        if with_bias:
            bq_cols = []
            for part in range(3):
                for h in hsel:
                    bq_cols.append(
                        bqkv[part * C + h * D : part * C + (h + 1) * D]
                    )
            bq_l = np.stack([np.concatenate(bq_cols[0:2]),
                             np.concatenate(bq_cols[2:4])])   # [2, 128] q,k
            bvv = np.concatenate(bq_cols[4:6]).reshape(PB, 1)  # [128,1] v

            def rot(vec):
                o = np.empty_like(vec)
                for base in (0, 64):
                    o[base:base + 32] = -vec[base + 32:base + 64]
                    o[base + 32:base + 64] = vec[base:base + 32]
                return o

            inv = 1.0 / (ROPE_BASE ** (np.arange(0, D, 2) / D))
            tt = np.arange(T, dtype=np.float32)
            fr = np.einsum("i,j->ij", tt, inv)
            emb = np.concatenate([fr, fr], axis=1)
            cosf = np.concatenate([np.cos(emb).T] * 2, axis=0).astype(np.float32)
            sinf = np.concatenate([np.sin(emb).T] * 2, axis=0).astype(np.float32)
            bt = np.empty((PB, 2, T), np.float32)
            for g in range(2):
                bcol = bq_l[g][:, None].astype(np.float32)
                bt[:, g, :] = bcol * cosf + rot(bcol[:, 0])[:, None] * sinf
            m["btbl"] = bt
            m["bv"] = np.ascontiguousarray(bvv.astype(np.float32))
        in_maps.append(m)

    key = with_bias
    if key not in _NC_CACHE:
        _NC_CACHE[key] = _build_nc(with_bias)
    res = run_bass_kernel_spmd(_NC_CACHE[key], in_maps,
                               core_ids=list(range(NCORES)))
    LAST_RESULTS = res
    acc = np.zeros((R, C), dtype=np.float32)
    for r in range(NCORES):
        acc += res.results[r]["out"].astype(np.float32)
    acc += bproj[None, :]
    return acc.reshape(B, T, C)
